# revision 1
# baseline (speedup 1.0000x reference)
"""Trainium2 Bass kernel for nn_BOW (bag-of-words MLP).

emb = relu(relu(relu(bow(idx) @ W1.T + b1) @ W2.T + b2) @ W3.T + b3)

Design (8 NeuronCores, ~67-73 us/core/call vs 210 us gather baseline):

fc1 is sharded over the vocab axis: core c owns rows [c*6272, (c+1)*6272) of
W1T (50000 padded to 50176).  Each core builds the bow histogram for ALL 256
batch rows restricted to its shard and multiplies it against the shard
(streamed from HBM once per call: 12.8 MB/core in bf16, vs 67 MB/core of
indirect-DMA gather traffic for the embedding-bag formulation).

Histogram without scatter: tokens are host-routed to (core, vocab-bucket of
128, batch-row-half) slots.  For each 128-token tile, one TensorE matmul
R^T @ M accumulates exact counts into PSUM, where R (token -> vocab position,
built on DVE by iota-compare) and M (token -> row, host-encoded fp8 0/1,
streamed) are one-hot matrices.  bowT then feeds the dense fc1 matmul as
lhsT with no transpose (vocab lands on partitions).  TensorE runs at full
128x128 MAC utilization for fc1; W1-shard DMA, one-hot DMA, and DVE one-hot
builds all overlap under it (single-pass bucket loop, 2-bucket software
pipeline, chunk-interleaved DMA issue).

Partial h1 [256, 1024] is exchanged with a single AllToAll (ReduceScatter is
~20x slower in this runtime) in bf16 and summed on TensorE with a
stacked-identity matmul whose output is ALREADY transposed (features on
partitions), so the bf16 fc2/fc3 tail needs no PE transposes; biases fold in
via per-partition activation bias or a ones-vector matmul.  A tiny
indirect-DMA gather path (128 slots) handles bucket-capacity overflow
exactly.  Host-side prep is index routing, one-hot encoding, dtype casts and
relayout only; all model arithmetic runs on device.
"""
import os, sys

os.environ.setdefault("JAX_PLATFORMS", "cpu,axon")
try:
    import concourse.bass  # noqa: F401
except ImportError:
    sys.path.insert(0, "/opt/trn_rl_repo")

import numpy as np
import concourse.bass as bass
import concourse.tile as tile
import concourse.mybir as mybir
from concourse import bacc
from concourse.bass_utils import run_bass_kernel_spmd
from concourse.masks import make_identity

N_CORES = 8
B, S = 256, 512
V = 50000
M1, M2, EMB = 1024, 512, 256
RPC = B // N_CORES           # rows per core after the exchange = 32
NB = 49                      # vocab buckets per core (49*128 = 6272)
VSH = NB * 128               # vocab shard size = 6272
P_B = 256                    # token slots per (bucket, row-half) cell
TPB = 4                      # tiles per bucket = 2 halves x 2 tiles
NT = NB * TPB                # token tiles per core = 196
CHT = 28                     # one-hot tiles per streamed chunk (196 = 7*28)
SPILL = 128                  # overflow token slots per core

_CACHE = {}


def _build(reps=1):
    nc = bacc.Bacc("TRN2", target_bir_lowering=False, debug=False, num_devices=N_CORES)
    f32 = mybir.dt.float32
    bf16 = mybir.dt.bfloat16
    f8 = mybir.dt.float8e4

    w1tl = nc.dram_tensor("w1tl", [128, NB, M1], bf16, kind="ExternalInput")
    w1s = nc.dram_tensor("w1s", [VSH, M1], bf16, kind="ExternalInput")
    rm = nc.dram_tensor("rm", [128, NT * 128], f8, kind="ExternalInput")
    rv = nc.dram_tensor("rv", [128, NT], mybir.dt.float32, kind="ExternalInput")
    msph = nc.dram_tensor("msph", [128, 256], f8, kind="ExternalInput")
    w2t = nc.dram_tensor("w2t", [128, M1 // 128, M2], bf16, kind="ExternalInput")
    w3t = nc.dram_tensor("w3t", [128, M2 // 128, EMB], bf16, kind="ExternalInput")
    b1tt = nc.dram_tensor("b1tt", [128, M1 // 128], mybir.dt.float32, kind="ExternalInput")
    b2tt = nc.dram_tensor("b2tt", [128, M2 // 128], mybir.dt.float32, kind="ExternalInput")
    b3r = nc.dram_tensor("b3r", [1, EMB], bf16, kind="ExternalInput")
    s4id = nc.dram_tensor("s4id", [128, RPC], bf16, kind="ExternalInput")
    spidx = nc.dram_tensor("spidx", [128, 1], mybir.dt.int32, kind="ExternalInput")
    emb = nc.dram_tensor("emb", [reps * RPC, EMB], f32, kind="ExternalOutput")

    with tile.TileContext(nc) as tc:
        with (
            tc.tile_pool(name="const", bufs=1) as cpool,
            tc.tile_pool(name="etab", bufs=1) as epool,
            tc.tile_pool(name="rmch", bufs=3) as rmpool,
            tc.tile_pool(name="bowsb", bufs=1) as bpool,
            tc.tile_pool(name="act", bufs=2) as apool,
            tc.tile_pool(name="oh", bufs=8) as ohpool,
            tc.tile_pool(name="pbow", bufs=2, space="PSUM") as pbow_pool,
            tc.tile_pool(name="ph1", bufs=1, space="PSUM") as ph1_pool,
            tc.tile_pool(name="ptail", bufs=1, space="PSUM") as ptail_pool,
            tc.tile_pool(name="dram", bufs=2, space="DRAM") as dpool,
        ):
            # ---- constants ----
            spidx_t = cpool.tile([128, 1], mybir.dt.int32)
            nc.sync.dma_start(spidx_t[:], spidx[:])
            msp = cpool.tile([128, 256], f8)
            nc.sync.dma_start(msp[:], msph[:])
            w2_t = cpool.tile([128, M1 // 128, M2], bf16)
            nc.sync.dma_start(w2_t[:], w2t[:])
            w3_t = cpool.tile([128, M2 // 128, EMB], bf16)
            nc.sync.dma_start(w3_t[:], w3t[:])
            b1_t = cpool.tile([128, M1 // 128], f32)
            nc.sync.dma_start(b1_t[:], b1tt[:])
            b2_t = cpool.tile([128, M2 // 128], f32)
            nc.sync.dma_start(b2_t[:], b2tt[:])
            b3_t = cpool.tile([1, EMB], bf16)
            nc.sync.dma_start(b3_t[:], b3r[:])
            s4_t = cpool.tile([128, RPC], bf16)
            nc.sync.dma_start(s4_t[:], s4id[:])
            rv_t = cpool.tile([128, NT], f32)
            nc.sync.dma_start(rv_t[:], rv[:])
            iota_i = cpool.tile([128, 128], mybir.dt.int32)
            nc.gpsimd.iota(iota_i[:], pattern=[[1, 128]], base=0, channel_multiplier=0)
            iotaR = cpool.tile([128, 128], bf16)
            nc.vector.tensor_copy(iotaR[:], iota_i[:])
            ones1 = cpool.tile([1, RPC], bf16)
            nc.gpsimd.memset(ones1[:], 1.0)
            gsp = cpool.tile([128, M1], bf16)
            nc.gpsimd.indirect_dma_start(
                out=gsp[:], out_offset=None, in_=w1s[:],
                in_offset=bass.IndirectOffsetOnAxis(ap=spidx_t[:, 0:1], axis=0),
            )

            for _rep in range(reps):
                _body(nc, tc, epool, rmpool, bpool, apool, ohpool,
                      pbow_pool, ph1_pool, ptail_pool, dpool,
                      gsp, msp, w2_t, w3_t, b1_t, b2_t, b3_t,
                      s4_t, ones1, rv_t, iotaR,
                      emb[_rep * RPC:(_rep + 1) * RPC, :], w1tl, rm)

    nc.compile()
    return nc


def _body(nc, tc, epool, rmpool, bpool, apool, ohpool,
          pbow_pool, ph1_pool, ptail_pool, dpool,
          gsp, msp, w2_t, w3_t, b1_t, b2_t, b3_t,
          s4_t, ones1, rv_t, iotaR, emb, w1tl, rm):
    f32 = mybir.dt.float32
    bf16 = mybir.dt.bfloat16
    f8 = mybir.dt.float8e4
    Relu = mybir.ActivationFunctionType.Relu
    Copy = mybir.ActivationFunctionType.Copy

    # ---- stream W1T shard + fp8 one-hots, interleaved in consumption
    # order: bucket group ci (7 buckets) needs rm chunk ci then et chunk ci ----
    et = epool.tile([128, NB, M1], bf16, tag="etab")
    CH = 7
    rmt = []
    for ci in range(NB // CH):
        rc = rmpool.tile([128, CHT * 128], f8, tag="rm")
        nc.sync.dma_start(rc[:], rm[:, ci * CHT * 128:(ci + 1) * CHT * 128])
        rmt.append(rc)
        for cj in range(CH):
            v = ci * CH + cj
            nc.sync.dma_start(et[:, v:v + 1, :], w1tl[:, v:v + 1, :])

    # ---- stage 1 (histogram matmul) + stage 2, single pass, 1-bucket
    # software pipeline so the DVE bowT copy hides under PE work ----
    bowT = bpool.tile([128, NB, 256], bf16, tag="bowT")
    ph1 = ph1_pool.tile([128, 2, 2, 512], f32, tag="h1")

    def stage2(q):
        for h in range(2):
            for b_ in range(2):
                nc.tensor.matmul(ph1[:, h, b_, :],
                                 lhsT=bowT[:, q, h * 128:(h + 1) * 128],
                                 rhs=et[:, q, b_ * 512:(b_ + 1) * 512],
                                 start=(q == 0), stop=False)

    for q in range(NB + 2):
        if q < NB:
            pb = pbow_pool.tile([128, 256], f32, tag="bow")
            for j in range(TPB):
                t = q * TPB + j
                h = j // 2
                rc = rmt[t // CHT]
                o = (t % CHT) * 128
                rt = ohpool.tile([128, 128], bf16, tag="oh")
                nc.vector.tensor_scalar(rt[:], iotaR[:], rv_t[:, t:t + 1], None,
                                        op0=mybir.AluOpType.is_equal)
                nc.tensor.matmul(pb[:, h * 128:(h + 1) * 128], lhsT=rt[:],
                                 rhs=rc[:, o:o + 128],
                                 start=(j % 2 == 0), stop=(j % 2 == 1))
            nc.vector.tensor_copy(bowT[:, q, :], pb[:])
        if q >= 2:
            stage2(q - 2)
    for h in range(2):
        for b_ in range(2):
            nc.tensor.matmul(ph1[:, h, b_, :], lhsT=msp[:, h * 128:(h + 1) * 128],
                             rhs=gsp[:, b_ * 512:(b_ + 1) * 512],
                             start=False, stop=(h == 1 and b_ == 1))
    h1p = apool.tile([128, 2, 2, 512], bf16, tag="h1p")
    for h in range(2):
        for b_ in range(2):
            nc.scalar.activation(h1p[:, h, b_, :], ph1[:, h, b_, :], Copy)

    # ---- exchange partial h1: AllToAll (8 chunks of 32 rows) ----
    cc_in = dpool.tile([B, M1], bf16, tag="cc_in")
    cc_out = dpool.tile([B, M1], bf16, tag="cc_out")
    nc.sync.dma_start(
        cc_in[:].rearrange("(h p) (b m) -> p h b m", p=128, b=2), h1p[:])
    nc.gpsimd.collective_compute(
        "AllToAll", mybir.AluOpType.bypass,
        replica_groups=[list(range(N_CORES))],
        ins=[cc_in[:]], outs=[cc_out[:]],
    )
    cc_sb = apool.tile([128, 2, M1], bf16, tag="ccsb")
    nc.sync.dma_start(
        cc_sb[:], cc_out[:].rearrange("(d q r) m -> (q r) d m", d=2, q=4))

    # ---- sum the 8 partials + b1 on TensorE, relu -> h1 [32, 1024] bf16 ----
    # ---- sum the 8 partials on TensorE, TRANSPOSED: h1T [feat128, a, rows] ----
    pt1 = ptail_pool.tile([128, M1 // 128, RPC], f32, tag="tail")
    for a in range(M1 // 128):
        for d in range(2):
            nc.tensor.matmul(pt1[:, a, :], lhsT=cc_sb[:, d, a * 128:(a + 1) * 128],
                             rhs=s4_t[:], start=(d == 0), stop=(d == 1))
    h1T = apool.tile([128, M1 // 128, RPC], bf16, tag="h1T")
    for a in range(M1 // 128):
        nc.scalar.activation(h1T[:, a, :], pt1[:, a, :], Relu,
                             bias=b1_t[:, a:a + 1])

    # ---- fc2, output transposed: h2T [feat128, m4, rows] ----
    pt2 = ptail_pool.tile([128, M1 // 128, RPC], f32, tag="tail")
    for m4 in range(M2 // 128):
        for a in range(M1 // 128):
            nc.tensor.matmul(pt2[:, m4, :], lhsT=w2_t[:, a, m4 * 128:(m4 + 1) * 128],
                             rhs=h1T[:, a, :],
                             start=(a == 0), stop=(a == M1 // 128 - 1))
    h2T = apool.tile([128, M2 // 128, RPC], bf16, tag="h2T")
    for m4 in range(M2 // 128):
        nc.scalar.activation(h2T[:, m4, :], pt2[:, m4, :], Relu,
                             bias=b2_t[:, m4:m4 + 1])

    # ---- fc3, row-major output [32, 256] ----
    pt3f = ptail_pool.tile([128, M1 // 128, RPC], f32, tag="tail")
    pt3 = pt3f[0:RPC, 0:EMB // RPC, :]
    for m4 in range(M2 // 128):
        nc.tensor.matmul(pt3, lhsT=h2T[:, m4, :], rhs=w3_t[:, m4, :],
                         start=(m4 == 0), stop=False)
    nc.tensor.matmul(pt3, lhsT=ones1[:], rhs=b3_t[:], start=False, stop=True)
    out_t = apool.tile([RPC, EMB], f32, tag="out")
    nc.scalar.activation(out_t[:], pt3, Relu)
    nc.sync.dma_start(emb[:], out_t[:])


def _prep_inputs(idx, W1, b1, W2, b2, W3, b3):
    """Host-side sharding/layout prep (index routing + dtype/layout only)."""
    import ml_dtypes

    bf16 = ml_dtypes.bfloat16
    f8np = mybir.dt.np(mybir.dt.float8e4)
    idx = np.asarray(idx).astype(np.int64)
    VPAD = N_CORES * VSH
    w1f = np.zeros((VPAD, M1), dtype=np.float32)
    w1f[:V] = np.asarray(W1, dtype=np.float32).T
    w1bf = w1f.astype(bf16)

    w2t = np.ascontiguousarray(
        np.asarray(W2, dtype=np.float32).T.reshape(M1 // 128, 128, M2)
        .transpose(1, 0, 2)).astype(bf16)
    w3t = np.ascontiguousarray(
        np.asarray(W3, dtype=np.float32).T.reshape(M2 // 128, 128, EMB)
        .transpose(1, 0, 2)).astype(bf16)
    b1tt = np.ascontiguousarray(
        np.asarray(b1, dtype=np.float32).reshape(M1 // 128, 128).T)
    b2tt = np.ascontiguousarray(
        np.asarray(b2, dtype=np.float32).reshape(M2 // 128, 128).T)
    b3r = np.asarray(b3, dtype=np.float32).reshape(1, EMB).astype(bf16)
    s4id = (np.arange(128)[:, None] % RPC == np.arange(RPC)[None, :]).astype(bf16)

    rows = np.repeat(np.arange(B, dtype=np.int64), S)
    vals = idx.reshape(-1)
    core = vals // VSH
    in_maps = []
    for c in range(N_CORES):
        sel = core == c
        v = vals[sel] - c * VSH
        r = rows[sel]
        q = v // 128
        rl = v % 128
        order = np.argsort(q, kind="stable")
        q, rl, r, v = q[order], rl[order], r[order], v[order]

        rv_arr = np.full((NT * 128,), 200, dtype=np.int64)
        rw_arr = np.full((NT * 128,), 300, dtype=np.int64)
        sp_idx = np.zeros((SPILL,), dtype=np.int32)
        sp_row = np.full((SPILL,), 300, dtype=np.int64)
        n_spill = 0
        for qq in range(NB):
            for hh in range(2):
                m = (q == qq) & ((r // 128) == hh)
                nq = int(m.sum())
                take = min(nq, P_B)
                base = (qq * 4 + hh * 2) * 128
                rv_arr[base:base + take] = rl[m][:take]
                rw_arr[base:base + take] = r[m][:take] % 128
                if nq > take:
                    ov = nq - take
                    assert n_spill + ov <= SPILL, "spill capacity exceeded"
                    sp_idx[n_spill:n_spill + ov] = v[m][take:]
                    sp_row[n_spill:n_spill + ov] = r[m][take:]
                    n_spill += ov
        rv_til = rv_arr.reshape(NT, 128).T        # [128, NT]
        rw_til = rw_arr.reshape(NT, 128).T
        rm_arr = (rw_til[:, :, None] == np.arange(128)[None, None, :]).astype(f8np)
        rm_arr = np.ascontiguousarray(rm_arr.reshape(128, NT * 128))
        mspa = (sp_row[:, None] == np.arange(256)[None, :]).astype(f8np)

        w1c = w1bf[c * VSH:(c + 1) * VSH]                     # [6272, 1024]
        w1tl = np.ascontiguousarray(
            w1c.reshape(NB, 128, M1).transpose(1, 0, 2))      # [128, 49, 1024]

        in_maps.append({
            "w1tl": w1tl,
            "w1s": np.ascontiguousarray(w1c),
            "rm": rm_arr,
            "rv": np.ascontiguousarray(rv_til.astype(np.float32)),
            "msph": mspa,
            "w2t": w2t, "w3t": w3t,
            "b1tt": b1tt, "b2tt": b2tt, "b3r": b3r, "s4id": s4id,
            "spidx": sp_idx.reshape(128, 1),
        })
    return in_maps


def kernel(idx, W1, b1, W2, b2, W3, b3):
    if "nc" not in _CACHE:
        _CACHE["nc"] = _build()
    nc = _CACHE["nc"]
    in_maps = _prep_inputs(idx, W1, b1, W2, b2, W3, b3)
    try:
        res = run_bass_kernel_spmd(nc, in_maps, list(range(N_CORES)))
    except Exception:
        res = run_bass_kernel_spmd(nc, in_maps, list(range(N_CORES)))
    return np.concatenate([res.results[c]["emb"] for c in range(N_CORES)], axis=0)

